# revision 1
# baseline (speedup 1.0000x reference)
"""Masked dot-product attention (B=2,H=16,L=2048,D=128) on 8 trn2 NeuronCores.

Strategy:
  - Shard batch*heads: core c handles (b=0,h=2c),(0,2c+1),(1,2c),(1,2c+1) -> 4 slots.
  - Per (b,h): compute S^T[k,q] = K Q^T directly on the PE (lhsT = k-tile
    transposed to [D,k], rhs = q transposed to [D,q]) so softmax masking is a
    per-partition bias on the exp eviction, and no P-transposes are needed.
  - Only ceil(valid_len/128) key tiles are computed (the rest contribute
    exactly 0 after exp of -1e9, matching the reference's mask fill).
  - exp is fused into the PSUM->SBUF eviction on the scalar engine with
    scale = 1/sqrt(D) and a per-partition -1e9 bias on the last partial tile.
  - O^T[d,q] += V_j^T P^T_j accumulates in PSUM (fp32), and the softmax
    denominator l[q] accumulates via an all-ones [128,1] lhsT matmul.
  - O^T is transposed back to [q,d] on the PE; the final eviction multiplies
    by 1/l per partition. Hot matmuls use float32r (~12-bit mantissa, 4x the
    fp32 PE throughput); accumulation stays fp32 in PSUM.
"""

import math
import os

import numpy as np

try:
    import concourse.bass as bass
except ImportError:  # pragma: no cover
    import sys

    sys.path.append("/opt/trn_rl_repo")
    import concourse.bass as bass

import concourse.mybir as mybir
import concourse.tile as tile
from concourse import bacc
from concourse.bass_utils import run_bass_kernel_spmd

B, H, L, D = 2, 16, 2048, 128
NCORES = 8
HPC = H // NCORES  # heads per core per batch
SLOTS = B * HPC  # bh slots per core
NEG = -1e9
INV_SQRT_D = 1.0 / math.sqrt(D)
F32 = mybir.dt.float32
F32R = mybir.dt.float32r
QT = L // 128  # 16 q tiles
QB = 4  # q blocks
QBW = L // QB  # 512 q per block
QTB = QT // QB  # 4 q tiles per block

_cache: dict = {}


def _build(K0: int, K1: int):
    """Build+compile the per-core program for K0/K1 valid key tiles."""
    Ks = [K0, K0, K1, K1]
    KM = max(K0, K1)
    nc = bacc.Bacc("TRN2", target_bir_lowering=False, debug=False, num_devices=NCORES)
    q = nc.dram_tensor("q", [SLOTS, L, D], F32R, kind="ExternalInput")
    k = nc.dram_tensor("k", [SLOTS, KM * 128, D], F32R, kind="ExternalInput")
    v = nc.dram_tensor("v", [SLOTS, KM * 128, D], F32R, kind="ExternalInput")
    identr = nc.dram_tensor("identr", [128, 128], F32R, kind="ExternalInput")
    identf = nc.dram_tensor("identf", [128, 128], F32, kind="ExternalInput")
    onesr = nc.dram_tensor("onesr", [128, 1], F32R, kind="ExternalInput")
    one1 = nc.dram_tensor("one1", [1, 1], F32, kind="ExternalInput")
    biases = nc.dram_tensor("biases", [128, SLOTS], F32, kind="ExternalInput")
    out = nc.dram_tensor("out", [SLOTS, L, D], F32, kind="ExternalOutput")

    with tile.TileContext(nc) as tc:
        with (
            tc.tile_pool(name="const", bufs=1) as constp,
            tc.tile_pool(name="io", bufs=2) as iop,
            tc.tile_pool(name="work", bufs=3) as workp,
            tc.tile_pool(name="ps", bufs=2, space="PSUM") as psp,
            tc.tile_pool(name="pso", bufs=2, space="PSUM") as psop,
        ):
            ident_r = constp.tile([128, 128], F32R)
            nc.sync.dma_start(out=ident_r, in_=identr[:, :])
            ident_f = constp.tile([128, 128], F32)
            nc.sync.dma_start(out=ident_f, in_=identf[:, :])
            ones_r = constp.tile([128, 1], F32R)
            nc.sync.dma_start(out=ones_r, in_=onesr[:, :])
            one_1 = constp.tile([1, 1], F32)
            nc.sync.dma_start(out=one_1, in_=one1[:, :])
            bias_sb = constp.tile([128, SLOTS], F32)
            nc.sync.dma_start(out=bias_sb, in_=biases[:, :])

            for s in range(SLOTS):
                Kv = Ks[s]
                qn = iop.tile([128, QT, 128], F32R, tag="qn")
                nc.sync.dma_start(out=qn, in_=q[s].rearrange("(t p) d -> p t d", p=128))
                kn = iop.tile([128, KM, 128], F32R, tag="kn")
                nc.sync.dma_start(
                    out=kn[:, :Kv, :],
                    in_=k[s, : Kv * 128, :].rearrange("(t p) d -> p t d", p=128),
                )
                vn = iop.tile([128, KM, 128], F32R, tag="vn")
                nc.sync.dma_start(
                    out=vn[:, :Kv, :],
                    in_=v[s, : Kv * 128, :].rearrange("(t p) d -> p t d", p=128),
                )

                # q -> qT [D, q] (grouped 4 transposes per PSUM bank + 1 evict)
                qTt = iop.tile([128, QT, 128], F32R, tag="qT")
                for g in range(QT // 4):
                    trp = psp.tile([128, 4, 128], F32R, tag="tr")
                    for ii in range(4):
                        nc.tensor.transpose(trp[:, ii, :], qn[:, g * 4 + ii, :], ident_r)
                    nc.scalar.copy(qTt[:, g * 4 : g * 4 + 4, :], trp)
                # k -> kT [D, k]
                kTt = iop.tile([128, KM, 128], F32R, tag="kT")
                for g in range((Kv + 3) // 4):
                    n = min(4, Kv - g * 4)
                    trp = psp.tile([128, 4, 128], F32R, tag="tr")
                    for ii in range(n):
                        nc.tensor.transpose(trp[:, ii, :], kn[:, g * 4 + ii, :], ident_r)
                    nc.scalar.copy(kTt[:, g * 4 : g * 4 + n, :], trp[:, :n, :])

                for qb in range(QB):
                    oT_ps = psop.tile([128, QBW], F32, tag="oT")
                    l_ps = psop.tile([1, QBW], F32, tag="l")
                    for j in range(Kv):
                        st_ps = psp.tile([128, QBW], F32, tag="st")
                        nc.tensor.matmul(
                            st_ps,
                            kTt[:, j, :],
                            qTt[:, qb * QTB : (qb + 1) * QTB, :],
                            start=True,
                            stop=True,
                        )
                        pT = workp.tile([128, QBW], F32R, tag="pT")
                        last = j == Kv - 1
                        nc.scalar.activation(
                            pT,
                            st_ps,
                            mybir.ActivationFunctionType.Exp,
                            bias=(bias_sb[:, s : s + 1] if last else 0.0),
                            scale=INV_SQRT_D,
                        )
                        nc.tensor.matmul(
                            oT_ps, vn[:, j, :], pT, start=(j == 0), stop=last
                        )
                        nc.tensor.matmul(
                            l_ps, ones_r, pT, start=(j == 0), stop=last
                        )

                    # finish this q block
                    oT_sb = workp.tile([128, QBW], F32, tag="oT_sb")
                    nc.vector.tensor_copy(oT_sb, oT_ps)
                    l_sb = workp.tile([1, QBW], F32, tag="l_sb")
                    nc.scalar.copy(l_sb, l_ps)
                    lt_ps = psp.tile([128, 4], F32, tag="tr")
                    for i in range(QTB):
                        nc.tensor.matmul(
                            lt_ps[:, i : i + 1],
                            l_sb[:, i * 128 : (i + 1) * 128],
                            one_1,
                            start=True,
                            stop=True,
                        )
                    lrec = workp.tile([128, 4], F32, tag="lrec")
                    nc.vector.reciprocal(lrec, lt_ps)
                    o_sb = workp.tile([128, QTB, 128], F32, tag="o_sb")
                    for i in range(QTB):
                        otr = psp.tile([128, 128], F32, tag="tr")
                        nc.tensor.transpose(otr, oT_sb[:, i * 128 : (i + 1) * 128], ident_f)
                        nc.vector.tensor_scalar_mul(o_sb[:, i, :], otr, lrec[:, i : i + 1])
                    nc.sync.dma_start(
                        out=out[s].rearrange("(t p) d -> p t d", p=128)[
                            :, qb * QTB : (qb + 1) * QTB, :
                        ],
                        in_=o_sb,
                    )
    nc.compile()
    return nc


def _get_program(K0: int, K1: int):
    key = (K0, K1)
    if key not in _cache:
        _cache[key] = _build(K0, K1)
    return _cache[key]


def _run(q, k, v, valid_lens, trace=False):
    q = np.ascontiguousarray(np.asarray(q, dtype=np.float32))
    k = np.ascontiguousarray(np.asarray(k, dtype=np.float32))
    v = np.ascontiguousarray(np.asarray(v, dtype=np.float32))
    vl = np.asarray(valid_lens).astype(np.int64)
    K0 = int(max(1, -(-vl[0] // 128)))
    K1 = int(max(1, -(-vl[1] // 128)))
    KM = max(K0, K1)
    nc = _get_program(K0, K1)

    # per-slot mask bias column: 0 for valid positions in the last key tile,
    # -1e9 beyond valid_len
    biases = np.zeros((128, SLOTS), dtype=np.float32)
    Ks = [K0, K0, K1, K1]
    bs = [0, 0, 1, 1]
    pos = np.arange(128)
    for s in range(SLOTS):
        rem = int(vl[bs[s]]) - (Ks[s] - 1) * 128
        biases[:, s] = np.where(pos < rem, 0.0, np.float32(NEG))

    identf = np.eye(128, dtype=np.float32)
    onesr = np.ones((128, 1), dtype=np.float32)
    one1 = np.ones((1, 1), dtype=np.float32)

    in_maps = []
    for c in range(NCORES):
        h0, h1 = 2 * c, 2 * c + 1
        qs = np.ascontiguousarray(
            np.stack([q[0, h0], q[0, h1], q[1, h0], q[1, h1]])
        )
        ks = np.ascontiguousarray(
            np.stack(
                [
                    k[0, h0, : KM * 128],
                    k[0, h1, : KM * 128],
                    k[1, h0, : KM * 128],
                    k[1, h1, : KM * 128],
                ]
            )
        )
        vs = np.ascontiguousarray(
            np.stack(
                [
                    v[0, h0, : KM * 128],
                    v[0, h1, : KM * 128],
                    v[1, h0, : KM * 128],
                    v[1, h1, : KM * 128],
                ]
            )
        )
        in_maps.append(
            {
                "q": qs,
                "k": ks,
                "v": vs,
                "identr": identf,
                "identf": identf,
                "onesr": onesr,
                "one1": one1,
                "biases": biases,
            }
        )

    res = run_bass_kernel_spmd(
        nc, in_maps, core_ids=list(range(NCORES)), trace=trace
    )

    outp = np.empty((B, H, L, D), dtype=np.float32)
    for c in range(NCORES):
        o = res.results[c]["out"]
        h0, h1 = 2 * c, 2 * c + 1
        outp[0, h0] = o[0]
        outp[0, h1] = o[1]
        outp[1, h0] = o[2]
        outp[1, h1] = o[3]
    return outp, res


def kernel(q, k, v, valid_lens):
    outp, _ = _run(q, k, v, valid_lens, trace=False)
    return outp
